# revision 19
# baseline (speedup 1.0000x reference)
"""Trainium2 Bass kernel v4 for nn_CamAttnCon (topk-masked CAM attention).

Strategy (per core, 4 samples, data-parallel over batch):
  - All constants (identity, tri, jcol, v2 row-indices, fore, tgt-fp16,
    ones) host-packed into ONE fp16 DMA on the ACT queue; emb shipped as 8
    half-sample DMAs on the SP queue (wire-limited ~5.8us).
  - num[t] / xn2[t] via PSUM-accumulating ap-1 PE matmuls over d-chunks
    (lhsT = emb/sq chunk, rhs = fore col / ones col) -> [128, TC] per
    sample, no DVE reduce.  Squares split per d-chunk: dc0/dc2 on DVE
    (fp16 2x), dc1/dc3 on ACT (Square).
  - Sqrt-free rank key v = numm*|numm|/xn2 (strictly monotone in cosine);
    the mask drives invalid lanes to ~-1e36 via numm = num - 1e18*m16n.
  - Selection via gpsimd kth_largest (k=50, q=0.9, n_valid = seqlen from
    the mask): out[0,1] is exactly the (m+1)-th largest f32 key, theta;
    sel = v > theta; gather weight recovered as g = sqrt(v_sel) on the
    compacted [J,1] tile in the gather's shadow.
  - Compaction: pos = TRI^T sel (PE) + chunk offsets from an exclusive
    cumsum (tensor_tensor_scan on a zero-shifted buffer); one-hot
    st = (pos*sel == j+1) against the materialized jvrep const (stride-1
    last dims -> DVE 2x); pst matmuls -> (row idx, v_sel) per slot;
    indirect-DMA gather of 52 att rows per sample.
  - tot[s] = sum_j g_j att[t_j, h, s] via PSUM-accumulating matmuls over
    h into [128, (g2, b)] (g2=1 covers s[68:196], fully-valid rows);
    PE-transpose to b-partition space; min/max + fused x*riv - mn*riv
    normalize on fp16 (DVE fast modes); [BL, 196] fp16 DMA out, f32 on host.
  - Emission software-pipelined in stage waves across samples so each
    engine queue stays in dependency order.
"""

import os
import sys

sys.path.insert(0, "/opt/trn_rl_repo")

import numpy as np
from contextlib import ExitStack

import concourse.bass as bass
import concourse.bacc as bacc
import concourse.mybir as mybir
import concourse.tile as tile
from concourse import bass_utils

f32 = mybir.dt.float32
fp16 = mybir.dt.float16
i32 = mybir.dt.int32
AX = mybir.AxisListType
OP = mybir.AluOpType
AF = mybir.ActivationFunctionType

B, T, D, H, S = 32, 512, 512, 8, 196
NCORES = 8
BL = B // NCORES            # 4 samples per core
TC = T // 128               # 4 t-chunks of 128
DC = D // 128               # 4 d-chunks of 128
HS = H * S                  # 1568
KK = 51                     # max top-k count
J = 52                      # padded slot count
NB = BL * TC                # 16

# const pack column offsets (fp16 [128, CW])
C_ID = 0            # id16 [128,128]
C_TRI = 128         # tri[p,q] = 1 if p<=q
C_V2 = 256          # v2[p,(b,c,2)]: col0 = b*T+c*128+p, col1 = 0 (g runtime)
C_FORE = 288        # fore[b, dc*128+p] at col b*DC+dc  [128,16]
C_TGT = 304         # tgt fp16 [128,16]
C_JVREP = 320       # jvrep[p, (j, c)] = j+1  [128, J*TC] (stride-1 last dim)
CW = 528

LAST_EXEC_NS = None
LAST_RESULTS = None


def build_body(ctx, tc_, emb, att, c16d, out):
    nc = tc_.nc

    const = ctx.enter_context(tc_.tile_pool(name="const", bufs=1))
    small = ctx.enter_context(tc_.tile_pool(name="small", bufs=1))
    embp = ctx.enter_context(tc_.tile_pool(name="embp", bufs=8))
    sqp = ctx.enter_context(tc_.tile_pool(name="sqp", bufs=int(os.environ.get("KSQP","8"))))
    gatp = ctx.enter_context(tc_.tile_pool(name="gatp", bufs=4))

    ps_nx = ctx.enter_context(tc_.tile_pool(name="ps_nx", bufs=1, space="PSUM"))
    ps_tot = ctx.enter_context(tc_.tile_pool(name="ps_tot", bufs=1, space="PSUM"))
    ps_sm = ctx.enter_context(tc_.tile_pool(name="ps_sm", bufs=1, space="PSUM"))

    # ---- phase 0: const DMA on ACT queue, warm sqrt table, emb on SP ----
    c16 = const.tile([128, CW], fp16, tag="c16")
    nc.scalar.dma_start(c16[:], c16d[:])

    warmf = small.tile([BL, S], f32, tag="warmf")
    if os.environ.get("ABL_CONST_OUT"):
        nc.vector.memset(warmf[:], 0.5)
    warm = small.tile([1, 1], f32, tag="warm")
    nc.vector.memset(warm[:], 1.0)
    warm2 = small.tile([1, 1], f32, tag="warm2")
    nc.scalar.sqrt(warm2[:], warm[:])

    zbuf = small.tile([1, 24], fp16, tag="zbuf")  # zrow [0:4], cr slots 4+5b..
    nc.vector.memset(zbuf[:], 0.0)

    embR = emb.rearrange("b (dc p) t -> b p dc t", p=128)
    embc = [[None] * DC for _ in range(BL)]  # embc[b][dc] = [128, T] fp16 AP
    for b in range(BL):
        for hh in range(2):
            if b == BL - 1 and hh == 1:
                # last sample's tail: quarter + two eighth DMAs so the final
                # squares start as early as possible
                e = embp.tile([128, T], fp16, tag="embq")
                nc.sync.dma_start(e[:], embR[b][:, 2, :])
                embc[b][2] = e[:]
                e3a = embp.tile([128, 256], fp16, tag="embe")
                nc.sync.dma_start(e3a[:], embR[b][:, 3, 0:256])
                e3b = embp.tile([128, 256], fp16, tag="embe")
                nc.sync.dma_start(e3b[:], embR[b][:, 3, 256:T])
                embc[b][3] = (e3a[:], e3b[:])
            else:
                e = embp.tile([128, 2 * T], fp16, tag="emb")
                nc.sync.dma_start(
                    e[:].rearrange("p (dc t) -> p dc t", dc=2),
                    embR[b][:, 2 * hh : 2 * hh + 2, :],
                )
                eRv = e[:].rearrange("p (dc t) -> p dc t", dc=2)
                embc[b][2 * hh] = eRv[:, 0, :]
                embc[b][2 * hh + 1] = eRv[:, 1, :]

    id16 = c16[:, C_ID : C_ID + 128]
    tri = c16[:, C_TRI : C_TRI + 128]
    jvrepR = c16[:, C_JVREP : C_JVREP + J * TC].rearrange("p (j c) -> p j c", c=TC)
    v2R = c16[:, C_V2 : C_V2 + 2 * NB].rearrange("p (b c two) -> p b c two", b=BL, c=TC)
    fore_c = c16[:, C_FORE : C_FORE + NB]
    tgt16 = c16[:, C_TGT : C_TGT + NB]
    # tri col 127 = (127 >= p) = all-ones column; tri row 0 = all-ones row
    ones_col = c16[:, C_TRI + 127 : C_TRI + 128]
    ones_row = c16[0:1, C_TRI : C_TRI + 128]

    # inverted seq mask: 1.0 where tgt <= 0 (invalid), with t=0 forced valid
    m16n = small.tile([128, NB], fp16, tag="m16n")
    nc.gpsimd.tensor_scalar(
        out=m16n[:], in0=tgt16[:], scalar1=0.0, scalar2=None, op0=OP.is_le
    )
    nc.gpsimd.memset(
        m16n[0:1, :].rearrange("p (b c) -> p b c", c=TC)[:, :, 0:1], 0.0
    )

    cidx = small.tile([J, 1], i32, tag="cidx")
    if os.environ.get("ABL_CONST_IDX"):
        nc.vector.memset(cidx[:], 7)

    # ---- per-sample state ----
    nx_ps = ps_nx.tile([128, 8 * BL], f32, tag="nx")  # cols b*8: 4 num, 4 xn2
    tot_ps = ps_tot.tile([128, 2 * BL], f32, tag="tot")  # col g2*BL + b

    def embt2(b, hh):
        # contiguous [128, 2T] view of half hh (valid except b3 h1)
        return embc[b][2 * hh].tensor.ap()[:, 0 : 2 * T]

    def num_rng(b):
        return nx_ps[:, b * 8 : b * 8 + TC]

    def xn2_rng(b):
        return nx_ps[:, b * 8 + TC : b * 8 + 2 * TC]

    sq = [[None] * DC for _ in range(BL)]
    xn = [None] * BL
    w32m = [None] * BL
    th = [None] * BL
    thc = [None] * BL
    sel = [None] * BL
    g16 = [None] * BL
    orow = [None] * BL
    pos_ps = [None] * BL
    posm = [None] * BL
    st = [None] * BL
    pst_ps = [None] * BL
    idxb = [None] * BL
    gsel = [None] * BL
    gat = [None] * BL

    def stage0(b):  # squares h0: b<3 dc0 DVE + dc1 ACT; b3 whole h0 on ACT
        if b == BL - 1:
            s_ = sqp.tile([128, 2 * T], fp16, tag="sqh")
            nc.scalar.activation(s_[:], embt2(b, 0), func=AF.Square)
            sq[b][0] = s_[:, 0:T]
            sq[b][1] = s_[:, T : 2 * T]
        else:
            s0 = sqp.tile([128, T], fp16, tag="sqq")
            if b == 2:
                nc.gpsimd.tensor_tensor(s0[:], embc[b][0], embc[b][0], op=OP.mult)
            else:
                nc.vector.tensor_tensor(s0[:], embc[b][0], embc[b][0], op=OP.mult)
            sq[b][0] = s0[:]
            s1 = sqp.tile([128, T], fp16, tag="sqq")
            nc.scalar.activation(s1[:], embc[b][1], func=AF.Square)
            sq[b][1] = s1[:]

    def echunk(b, dc, c):
        src = embc[b][dc]
        if isinstance(src, tuple):
            return src[c // 2][:, (c % 2) * 128 : (c % 2) * 128 + 128]
        return src[:, c * 128 : (c + 1) * 128]

    def sqchunk(b, dc, c):
        src = sq[b][dc]
        if isinstance(src, tuple):
            return src[c // 2][:, (c % 2) * 128 : (c % 2) * 128 + 128]
        return src[:, c * 128 : (c + 1) * 128]

    def stage1(b):  # squares h1 (b<3: dc2 DVE + dc3 ACT; b3: both DVE) + mms
        for dc in (2, 3):
            # b0/b1: ACT takes dc2+dc3 (ACT is idle mid-window); b2: dc3 on
            # ACT; b3: both quarters on DVE (ACT busy with its h0)
            on_act = (b <= 1) or (b == 2 and dc == 3)
            if isinstance(embc[b][dc], tuple):
                sa = sqp.tile([128, 256], fp16, tag="sqe")
                nc.vector.tensor_tensor(sa[:], embc[b][dc][0], embc[b][dc][0], op=OP.mult)
                sb_ = sqp.tile([128, 256], fp16, tag="sqe")
                nc.vector.tensor_tensor(sb_[:], embc[b][dc][1], embc[b][dc][1], op=OP.mult)
                sq[b][dc] = (sa[:], sb_[:])
            else:
                s_ = sqp.tile([128, T], fp16, tag="sqq")
                if on_act:
                    nc.scalar.activation(s_[:], embc[b][dc], func=AF.Square)
                else:
                    nc.vector.tensor_tensor(s_[:], embc[b][dc], embc[b][dc], op=OP.mult)
                sq[b][dc] = s_[:]
        for c in range(TC):
            for dc in range(DC):
                nc.tensor.matmul(
                    out=nx_ps[:, b * 8 + c : b * 8 + c + 1],
                    lhsT=echunk(b, dc, c),
                    rhs=fore_c[:, b * DC + dc : b * DC + dc + 1],
                    start=(dc == 0), stop=(dc == DC - 1),
                )
        for c in range(TC):
            for dc in range(DC):
                nc.tensor.matmul(
                    out=nx_ps[:, b * 8 + TC + c : b * 8 + TC + c + 1],
                    lhsT=sqchunk(b, dc, c),
                    rhs=ones_col,
                    start=(dc == 0), stop=(dc == DC - 1),
                )

    def stage2(b):  # sqrt-free rank key v = numm*|numm|/xn2 + kth_largest
        numm = small.tile([128, TC], f32, tag=f"numm{b}")
        nc.vector.scalar_tensor_tensor(
            out=numm[:], in0=m16n[:].rearrange("p (b c) -> p b c", b=BL)[:, b, :],
            scalar=-1e18, in1=num_rng(b), op0=OP.mult, op1=OP.add,
        )
        a2 = small.tile([128, TC], f32, tag=f"a2{b}")
        if os.environ.get("KABS", "1") == "1":
            nc.scalar.activation(a2[:], numm[:], func=AF.Abs)
        else:
            ng = small.tile([128, TC], f32, tag=f"ng{b}")
            nc.vector.tensor_scalar(
                out=ng[:], in0=numm[:], scalar1=-1.0, scalar2=None, op0=OP.mult
            )
            nc.vector.tensor_tensor(a2[:], numm[:], ng[:], op=OP.max)
        rx2 = small.tile([128, TC], f32, tag=f"rx2{b}")
        nc.vector.reciprocal(rx2[:], xn2_rng(b))
        v1 = small.tile([128, TC], f32, tag=f"v1{b}")
        nc.vector.tensor_tensor(v1[:], numm[:], a2[:], op=OP.mult)
        wm = small.tile([128, TC], f32, tag=f"vm{b}")
        nc.vector.tensor_tensor(wm[:], v1[:], rx2[:], op=OP.mult)
        w32m[b] = wm
        th_b = small.tile([1, 2], f32, tag=f"th{b}")
        nc.gpsimd.kth_largest(th_b[:], wm[:], n_per_lane=TC, k=KK - 1, quantile=0.9)
        th[b] = th_b
        thc_b = small.tile([128, 1], f32, tag=f"thc{b}")
        nc.gpsimd.partition_broadcast(thc_b[:], th_b[0:1, 1:2], channels=128)
        thc[b] = thc_b

    def stage3(b):  # sel, g, ctot + pos matmuls, scan
        sel_b = small.tile([128, TC], fp16, tag=f"sel{b}")
        nc.vector.tensor_scalar(
            out=sel_b[:], in0=w32m[b][:], scalar1=thc[b][:, 0:1], scalar2=None,
            op0=OP.is_gt,
        )
        sel[b] = sel_b
        g_b = small.tile([128, TC], fp16, tag=f"gv{b}")
        nc.vector.tensor_tensor(g_b[:], w32m[b][:], sel_b[:], op=OP.mult)
        g16[b] = g_b
        ct_ps = ps_sm.tile([1, TC], f32, tag="sm", bufs=int(os.environ.get("KSM","1")))
        nc.tensor.matmul(out=ct_ps[:], lhsT=ones_col, rhs=sel_b[:], start=True, stop=True)
        p_ps = ps_sm.tile([128, TC], f32, tag="pos", bufs=int(os.environ.get("KPOS","3")))
        nc.tensor.matmul(out=p_ps[:], lhsT=tri, rhs=sel_b[:], start=True, stop=False)
        pos_ps[b] = p_ps
        # inclusive scan of chunk totals written at 5b+1 -> [5b..5b+3] is the
        # exclusive prefix (5b is a permanent zero)
        nc.vector.tensor_tensor_scan(
            zbuf[0:1, 4 + 5 * b + 1 : 4 + 5 * b + 5], ct_ps[:], zbuf[0:1, 0:4],
            initial=0.0, op0=OP.add, op1=OP.add,
        )

    def stage4(b):  # pos += offsets; posm; one-hot st
        nc.tensor.matmul(
            out=pos_ps[b][:], lhsT=ones_row,
            rhs=zbuf[0:1, 4 + 5 * b : 4 + 5 * b + 4], start=False, stop=True
        )
        pm = small.tile([128, TC], fp16, tag=f"posm{b}")
        nc.vector.tensor_tensor(pm[:], pos_ps[b][:], sel[b][:], op=OP.mult)
        posm[b] = pm
        # write g into v2 col 1 for this sample
        st_b = small.tile([128, J * TC], fp16, tag=f"st{b}")
        nc.vector.tensor_tensor(
            st_b[:].rearrange("p (j c) -> p j c", j=J),
            pm[:].unsqueeze(1).broadcast_to([128, J, TC]),
            jvrepR,
            op=OP.is_equal,
        )
        st[b] = st_b

    def stage5(b):  # pst matmuls (idx group then g group) -> idx + gsel
        stR = st[b][:].rearrange("p (j c) -> p j c", j=J)
        pp = ps_sm.tile([J, 2], f32, tag="pst", bufs=2)
        for c in range(TC):
            nc.tensor.matmul(
                out=pp[:, 0:1], lhsT=stR[:, :, c], rhs=v2R[:, b, c, 0:1],
                start=(c == 0), stop=(c == TC - 1),
            )
        for c in range(TC):
            nc.tensor.matmul(
                out=pp[:, 1:2], lhsT=stR[:, :, c], rhs=g16[b][:, c : c + 1],
                start=(c == 0), stop=(c == TC - 1),
            )
        pst_ps[b] = pp
        ix = small.tile([J, 1], i32, tag=f"idx{b}")
        nc.scalar.copy(ix[:], pp[:, 0:1])
        idxb[b] = ix

    def stage6(b):  # gather; g = sqrt(v_sel) on ACT in the gather's shadow
        gt = gatp.tile([J, HS], fp16, tag="gat")
        _off = cidx[:, 0:1] if os.environ.get("ABL_CONST_IDX") else idxb[b][:, 0:1]
        nc.gpsimd.indirect_dma_start(
            out=gt[:], out_offset=None, in_=att[:],
            in_offset=bass.IndirectOffsetOnAxis(ap=_off, axis=0),
        )
        gat[b] = gt
        gs = small.tile([J, 1], fp16, tag=f"gsel{b}")
        nc.scalar.activation(gs[:], pst_ps[b][:, 1:2], func=AF.Sqrt)
        gsel[b] = gs

    def stage7(b):  # tot matmuls, accumulate over h
        # g2=1 block covers s[68:196] (full 128 rows; s68..127 duplicated so
        # the PSUM tile has no uninitialized rows and the tail needs no memset)
        for g2 in range(2):
            off = 0 if g2 == 0 else S - 128
            for h in range(H):
                nc.tensor.matmul(
                    out=tot_ps[:, g2 * BL + b : g2 * BL + b + 1],
                    lhsT=gat[b][:, h * S + off : h * S + off + 128],
                    rhs=gsel[b][:],
                    start=(h == 0), stop=(h == H - 1),
                )

    stages = [stage0, stage1, stage2, stage3, stage4, stage5, stage6, stage7]
    NS = len(stages)
    # emission order approximates each (stage, sample)'s ready time in ~0.1us
    # units: data arrival per sample + cumulative chain offset per stage
    DT = [int(x) for x in os.environ.get("KDT", "0,15,29,51").split(",")]
    CO = [int(x) for x in os.environ.get("KCO", "0,7,10,14,17,20,26,90").split(",")]
    order = sorted(
        ((s, b) for s in range(NS) for b in range(BL)),
        key=lambda sb: (DT[sb[1]] + CO[sb[0]], sb[1]),
    )
    if os.environ.get("KG32", "0") == "1":
        # pool runs its stream in order: put b3's gather gen ahead of b2's so
        # the critical sample's gather isn't delayed by b2's 1012ns desc-gen
        order.remove((6, 2))
        order.insert(order.index((6, 3)) + 1, (6, 2))
    hp = os.environ.get("KHP", "0") == "1"
    for s_i, b in order:
        if hp and b == BL - 1 and 2 <= s_i <= 6:
            # critical-path-first: the last sample's selection chain ops are
            # preferred by the scheduler the moment they become ready
            with tc_.high_priority():
                stages[s_i](b)
        else:
            stages[s_i](b)

    # ---- tail: transpose to b-partition space, normalize, one DMA out ----
    totsb = small.tile([128, 2 * BL], fp16, tag="totsb")
    nc.scalar.copy(totsb[:], tot_ps[:])
    tps0 = ps_sm.tile([BL, 128], fp16, tag="pos", bufs=int(os.environ.get("KPOS","3")))
    nc.tensor.transpose(tps0[:], totsb[:, 0:BL], id16)
    tps1 = ps_sm.tile([BL, 128], fp16, tag="pos", bufs=int(os.environ.get("KPOS","3")))
    nc.tensor.transpose(tps1[:], totsb[:, BL : 2 * BL], id16)
    outsb = small.tile([BL, 256], fp16, tag="outsb")
    nc.vector.tensor_copy(outsb[:, 0:128], tps0[:])
    nc.scalar.copy(outsb[:, 128:S], tps1[:, 128 - (S - 128) : 128])
    mn = small.tile([BL, 1], f32, tag="mn")
    nc.vector.tensor_reduce(mn[:].unsqueeze(2), outsb[:, 0:S].unsqueeze(1), axis=AX.X, op=OP.min)
    mx = small.tile([BL, 1], f32, tag="mx")
    nc.vector.tensor_reduce(mx[:].unsqueeze(2), outsb[:, 0:S].unsqueeze(1), axis=AX.X, op=OP.max)
    rngc = small.tile([BL, 1], f32, tag="rngc")
    nc.vector.tensor_scalar(
        out=rngc[:], in0=mx[:], scalar1=mn[:, 0:1], scalar2=1e-12,
        op0=OP.subtract, op1=OP.max,
    )
    riv = small.tile([BL, 1], f32, tag="riv")
    nc.vector.reciprocal(riv[:], rngc[:])
    # (x - mn) * riv directly: two per-partition scalars applied in sequence,
    # no broadcast operand, fp16 tensors -> DVE fast mode
    outf = small.tile([BL, 256], fp16, tag="outf")
    nc.vector.tensor_scalar(
        out=outf[:, 0:S], in0=outsb[:, 0:S], scalar1=mn[:, 0:1],
        scalar2=riv[:, 0:1], op0=OP.subtract, op1=OP.mult,
    )
    if os.environ.get("ABL_CONST_OUT"):
        nc.sync.dma_start(out[:], warmf[0:BL, 0:S])
    elif os.environ.get("KOUTSW", "0") == "1":
        nc.gpsimd.dma_start(out[:], outf[:, 0:S])
    else:
        nc.sync.dma_start(out[:], outf[:, 0:S])


def build_nc(path=None):
    nc = bacc.Bacc("TRN2", target_bir_lowering=False, debug=False)
    emb = nc.dram_tensor("emb", [BL, D, T], fp16, kind="ExternalInput")
    att = nc.dram_tensor("att", [BL * T, HS], fp16, kind="ExternalInput")
    c16d = nc.dram_tensor("c16", [128, CW], fp16, kind="ExternalInput")
    out = nc.dram_tensor("out", [BL, S], fp16, kind="ExternalOutput")
    with ExitStack() as ctx:
        tc_ = ctx.enter_context(tile.TileContext(nc))
        build_body(ctx, tc_, emb.ap(), att.ap(), c16d.ap(), out.ap())
    nc.compile()
    return nc


_NC_CACHE = {}


def get_nc(path=None):
    if "nc" not in _NC_CACHE:
        _NC_CACHE["nc"] = build_nc()
    return _NC_CACHE["nc"]


def make_consts():
    c = np.zeros((128, CW), dtype=np.float16)
    c[:, C_ID : C_ID + 128] = np.eye(128, dtype=np.float16)
    q = np.arange(128)
    c[:, C_TRI : C_TRI + 128] = (q[None, :] >= q[:, None]).astype(np.float16)
    c[:, C_JVREP : C_JVREP + J * TC] = np.repeat(
        (np.arange(J) + 1).astype(np.float16), TC
    )[None, :]
    v2 = np.zeros((128, BL, TC, 2), dtype=np.float16)
    v2[:, :, :, 0] = (
        np.arange(BL)[None, :, None] * T
        + np.arange(TC)[None, None, :] * 128
        + q[:, None, None]
    )
    c[:, C_V2 : C_V2 + 2 * NB] = v2.reshape(128, 2 * NB)
    return c


def make_in_maps(fore_rep_encoded, target_embed, align_attns, targets):
    LAYER_ID = 2
    att_l = np.transpose(np.asarray(align_attns[LAYER_ID]), (0, 2, 1, 3))  # [B,T,H,S]
    att_l = np.ascontiguousarray(att_l, dtype=np.float16)
    emb_d = np.ascontiguousarray(
        np.swapaxes(np.asarray(target_embed), 1, 2), dtype=np.float16
    )  # [B, D, T]
    fore_np = np.asarray(fore_rep_encoded, dtype=np.float16)  # [B, D]
    tgt_np = np.asarray(targets)[:, :T].astype(np.float32)    # [B, T]
    cbase = make_consts()
    in_maps = []
    for cidx in range(NCORES):
        sl = slice(cidx * BL, (cidx + 1) * BL)
        c = cbase.copy()
        fore_sl = fore_np[sl]                      # [BL, D]
        c[:, C_FORE : C_FORE + NB] = (
            fore_sl.reshape(BL, DC, 128).transpose(2, 0, 1).reshape(128, NB)
        )
        tgt_sl = tgt_np[sl]                        # [BL, T]
        c[:, C_TGT : C_TGT + NB] = (
            tgt_sl.reshape(BL, TC, 128).transpose(2, 0, 1).reshape(128, NB)
        ).astype(np.float16)
        in_maps.append(
            {
                "emb": np.ascontiguousarray(emb_d[sl]),
                "att": att_l[sl].reshape(BL * T, HS),
                "c16": c,
            }
        )
    return in_maps


def kernel(fore_rep_encoded, target_embed, align_attns, targets):
    global LAST_EXEC_NS, LAST_RESULTS
    nc = get_nc()
    in_maps = make_in_maps(fore_rep_encoded, target_embed, align_attns, targets)
    trace = bool(os.environ.get("KERNEL_TRACE"))
    try:
        res = bass_utils.run_bass_kernel_spmd(
            nc, in_maps, core_ids=list(range(NCORES)), trace=trace
        )
    except ModuleNotFoundError:
        os.environ["BASS_NEVER_TRACE"] = "1"
        res = bass_utils.run_bass_kernel_spmd(
            nc, in_maps, core_ids=list(range(NCORES)), trace=False
        )
    LAST_EXEC_NS = res.exec_time_ns
    LAST_RESULTS = res
    return np.concatenate([r["out"] for r in res.results], axis=0).astype(np.float32)
